# revision 20
# baseline (speedup 1.0000x reference)
"""MLAttention (label-pooling attention) Trainium2 Bass kernel.

Computes, for full inputs:
    scores = einsum('bsh,lh->bls', inputs, W)
    scores = where(mask==0, -inf, scores)
    attn   = softmax(scores, axis=-1)
    out    = einsum('bls,bsh->blh', attn, inputs)

Label-parallel across 8 NeuronCores: L=28415 padded to 28672 = 8*3584.
Each core gets its own W shard [3584, 512]; inputs/masks replicated.
Host concatenates the 8 per-core outputs [B, 3584, H] and trims to L.

Variant "c" (current): transposed-scores dataflow sized so the PE runs
*only* the 896 essential N=512 matmuls per core (no transposes, no
rowsum matmuls):

  mm1:  scoresT[s,l] psum = sum_k XT_k^T @ WT_k   (16 MMs per 512-label
        group; stationary = XT chunk, moving = W labels)
  ACT:  exp_g = Exp(scoresT + mask_bias)          (bias per-partition = per-s)
  DVE:  tot   = sum_sc exp_g chunks               (3 adds)
  GPS:  totb  = partition_all_reduce(tot)         (softmax denom, bcast)
  DVE:  attn  = exp_g * reciprocal(totb)          (f32r, pre-normalized)
  mm2:  out[l,h] psum = sum_sc attn_sc^T @ XB_sc  (16 MMs per group;
        stationary = attn chunk -- directly, no transpose needed)
  evac: ACT/DVE copy psum->SBUF, DMA to out[b, l-tile, :]

One-group software pipeline (mm2 of group i emitted after mm1 of group
i+1) hides the exp->normalize chain latency behind mm1 of the next
group, so the PE never waits on softmax.

Matmul operands are float32r end-to-end: the DRAM tensors are declared
f32r so DMA lands input bytes directly in the resident SBUF tiles (no
staging casts). A short burst of dummy matmuls at t=0 warms the PE HAM
clock gate (1.2 -> 2.4 GHz) while the first input DMAs stream.
"""

from contextlib import ExitStack

import numpy as np

import concourse.bass as bass
import concourse.mybir as mybir
import concourse.tile as tile
from concourse import bacc, bass_utils
from concourse.bass import bass_isa, ds, ts

F32 = mybir.dt.float32
R32 = mybir.dt.float32r

# Problem shapes (hardcoded per contract).
B, S, H, L = 4, 512, 512, 28415
N_CORES = 8
LSH = 3584               # per-core padded label count
L_PAD = LSH * N_CORES    # 28672


def build_module_c(b_sz=B, s_sz=S, h_sz=H, lsh=LSH, n_devices=N_CORES,
                   n_warm=24):
    P = 128
    KH = h_sz // P   # H contraction chunks (mm1)
    KS = s_sz // P   # S contraction chunks (mm2)
    LG = 512         # label group per pass (PSUM bank limit)
    NG = lsh // LG   # label groups per core
    NSUB = LG // P   # 128-label output tiles per group

    nc = bacc.Bacc(
        "TRN2", target_bir_lowering=False, debug=False, num_devices=n_devices
    )
    x_d = nc.dram_tensor("x", [b_sz, s_sz, h_sz], R32, kind="ExternalInput").ap()
    xt_d = nc.dram_tensor("xt", [b_sz, h_sz, s_sz], R32, kind="ExternalInput").ap()
    wt_d = nc.dram_tensor("wt", [h_sz, lsh], R32, kind="ExternalInput").ap()
    m_d = nc.dram_tensor("m", [b_sz, s_sz], F32, kind="ExternalInput").ap()
    o_d = nc.dram_tensor("o", [b_sz, lsh, h_sz], F32, kind="ExternalOutput").ap()

    with tile.TileContext(nc) as tc, ExitStack() as ctx:
        const = ctx.enter_context(tc.tile_pool(name="const", bufs=1))
        res = ctx.enter_context(tc.tile_pool(name="res", bufs=1))
        work = ctx.enter_context(tc.tile_pool(name="work", bufs=2))
        psum = ctx.enter_context(tc.tile_pool(name="psum", bufs=1, space="PSUM"))

        warm_f = const.tile([P, LG], F32)
        nc.gpsimd.memset(warm_f[:], 0.0)
        warm_w = const.tile([P, P], R32)
        nc.vector.tensor_copy(warm_w[:], warm_f[:, :P])
        warm_x = const.tile([P, LG], R32)
        nc.vector.tensor_copy(warm_x[:], warm_f[:])

        # Resident SBUF tensors; DMA'd straight from DRAM (same bits).
        WT = res.tile([P, KH, lsh], R32)          # WT[h%128, h//128, l] = W[l, h]
        XT = res.tile([P, b_sz, KH, s_sz], R32)   # XT[h%128, b, h//128, s]
        XB = res.tile([P, b_sz, KS, h_sz], R32)   # XB[s%128, b, s//128, h]
        MB = res.tile([P, b_sz, KS], F32)         # exp bias: (mask-1)*30 per s

        def mask_setup():
            mbr = work.tile([P, b_sz, KS], F32, tag="mbr")
            nc.sync.dma_start(mbr[:], m_d.rearrange("b (c p) -> p b c", p=P))
            nc.vector.tensor_scalar_mul(out=mbr[:], in0=mbr[:], scalar1=30.0)
            nc.vector.tensor_scalar_add(out=MB[:], in0=mbr[:], scalar1=-30.0)

        def warmup(n):
            ps_warm = psum.tile([P, LG], F32, tag="ps_out", bufs=4)
            for _ in range(n):
                nc.tensor.matmul(
                    ps_warm[:], warm_w[:], warm_x[:], start=True, stop=True
                )

        def dma_xt(b):
            nc.sync.dma_start(XT[:, b], xt_d[b].rearrange("(k p) s -> p k s", p=P))

        def dma_xb(b):
            nc.sync.dma_start(XB[:, b], x_d[b].rearrange("(c p) h -> p c h", p=P))

        def dma_wt(g):
            nc.sync.dma_start(
                WT[:, :, ts(g, LG)],
                wt_d[:, ts(g, LG)].rearrange("(k p) l -> p k l", p=P),
            )

        def mm1(b, g):
            ps_sct = psum.tile([P, KS, LG], F32, tag="ps_sct", bufs=1)
            for sc in range(KS):
                for k in range(KH):
                    nc.tensor.matmul(
                        ps_sct[:, sc, :],
                        XT[:, b, k, ts(sc, P)],
                        WT[:, k, ts(g, LG)],
                        start=(k == 0),
                        stop=(k == KH - 1),
                    )
            return ps_sct

        def exp_front(b, g, ps_sct):
            """exp into f32r -- directly the mm2 stationary."""
            exp_g = work.tile([P, KS, LG], R32, tag="exp", bufs=3)
            for sc in range(KS):
                nc.scalar.activation(
                    exp_g[:, sc, :], ps_sct[:, sc, :],
                    mybir.ActivationFunctionType.Exp,
                    bias=MB[:, b, sc : sc + 1],
                )
            return exp_g

        def rowsum(exp_g):
            """Label-sums: DVE adds + gpsimd partition reduce, then a tiny
            SBUF->SBUF DMA lays the row out as a per-label column. Two
            pipeline stages of slack before mm2's evac scale needs it."""
            tot = work.tile([P, LG], F32, tag="tot", bufs=2)
            nc.vector.tensor_tensor(
                out=tot[:], in0=exp_g[:, 0, :], in1=exp_g[:, 1, :],
                op=mybir.AluOpType.add,
            )
            nc.vector.tensor_tensor(
                out=tot[:], in0=tot[:], in1=exp_g[:, 2, :],
                op=mybir.AluOpType.add,
            )
            nc.vector.tensor_tensor(
                out=tot[:], in0=tot[:], in1=exp_g[:, 3, :],
                op=mybir.AluOpType.add,
            )
            nc.gpsimd.partition_all_reduce(
                tot[:], tot[:], channels=P, reduce_op=bass_isa.ReduceOp.add
            )
            sums_col = work.tile([P, NSUB], F32, tag="sums_col", bufs=2)
            for l in range(NSUB):
                nc.sync.dma_start(sums_col[:, l : l + 1], tot[0:1, ts(l, P)])
            return sums_col

        def mm2(b, g, exp_g, sums_col):
            recips = work.tile([P, NSUB], F32, tag="recips", bufs=2)
            nc.vector.reciprocal(recips[:], sums_col[:])

            for l in range(NSUB):
                ps_out = psum.tile([P, h_sz], F32, tag="ps_out", bufs=4)
                for sc in range(KS):
                    nc.tensor.matmul(
                        ps_out[:],
                        exp_g[:, sc, ts(l, P)],
                        XB[:, b, sc, :],
                        start=(sc == 0),
                        stop=(sc == KS - 1),
                    )
                out_t = work.tile([P, h_sz], F32, tag="out", bufs=6)
                # ACT-evacuated tiles go out on the scalar-issued DMA queue,
                # DVE-evacuated ones on the sync queue: no single DMA FIFO
                # gets head-of-line blocked by a late producer.
                if l < 2:
                    nc.scalar.activation(
                        out_t[:], ps_out[:], mybir.ActivationFunctionType.Copy,
                        scale=recips[:, l : l + 1],
                    )
                    nc.scalar.dma_start(
                        o_d[b, ds(g * LG + l * P, P), :], out_t[:]
                    )
                else:
                    nc.vector.tensor_scalar_mul(
                        out=out_t[:], in0=ps_out[:],
                        scalar1=recips[:, l : l + 1],
                    )
                    nc.sync.dma_start(
                        o_d[b, ds(g * LG + l * P, P), :], out_t[:]
                    )

        warmup(n_warm)
        mask_setup()
        dma_xt(0)
        dma_wt(0)
        dma_wt(1)
        dma_xb(0)
        pend = []
        for b in range(b_sz):
            for g in range(NG):
                if b == 0 and g + 2 < NG:
                    dma_wt(g + 2)
                if g == 3 and b + 1 < b_sz:
                    dma_xt(b + 1)
                    dma_xb(b + 1)
                ps = mm1(b, g)
                exp_g = exp_front(b, g, ps)
                if len(pend) == 2:
                    mm2(*pend.pop(0))
                sums_col = rowsum(exp_g)
                pend.append((b, g, exp_g, sums_col))
        for args in pend:
            mm2(*args)

    nc.compile()
    return nc


_CACHE = {}

VARIANT = "c"


def _get_module():
    if VARIANT not in _CACHE:
        _CACHE[VARIANT] = build_module_c()
    return _CACHE[VARIANT]


def _run(inputs: np.ndarray, masks: np.ndarray, W: np.ndarray, **spmd_kwargs):
    """Run on 8 cores; returns (full output, BassKernelResults)."""
    nc = _get_module()

    x = np.ascontiguousarray(inputs, dtype=np.float32)
    xt = np.ascontiguousarray(np.swapaxes(x, 1, 2))
    mf = np.ascontiguousarray(masks, dtype=np.float32)
    wt_pad = np.zeros((H, L_PAD), dtype=np.float32)
    wt_pad[:, :L] = W.T

    in_maps = [
        {
            "x": x,
            "xt": xt,
            "m": mf,
            "wt": np.ascontiguousarray(wt_pad[:, c * LSH : (c + 1) * LSH]),
        }
        for c in range(N_CORES)
    ]
    res = bass_utils.run_bass_kernel_spmd(
        nc, in_maps, core_ids=list(range(N_CORES)), **spmd_kwargs
    )
    out = np.concatenate([res.results[c]["o"] for c in range(N_CORES)], axis=1)
    return np.ascontiguousarray(out[:, :L, :]), res


def kernel(inputs: np.ndarray, masks: np.ndarray, W: np.ndarray) -> np.ndarray:
    out, _ = _run(inputs, masks, W)
    return out


# revision 23
# speedup vs baseline: 1.0040x; 1.0040x over previous
"""MLAttention (label-pooling attention) Trainium2 Bass kernel.

Computes, for full inputs:
    scores = einsum('bsh,lh->bls', inputs, W)
    scores = where(mask==0, -inf, scores)
    attn   = softmax(scores, axis=-1)
    out    = einsum('bls,bsh->blh', attn, inputs)

Label-parallel across 8 NeuronCores: L=28415 padded to 28672 = 8*3584.
Each core gets its own W shard [3584, 512]; inputs/masks replicated.
Host concatenates the 8 per-core outputs [B, 3584, H] and trims to L.

Variant "c" (current): transposed-scores dataflow sized so the PE runs
*only* the 896 essential N=512 matmuls per core (no transposes, no
rowsum matmuls):

  mm1:  scoresT[s,l] psum = sum_k XT_k^T @ WT_k   (16 MMs per 512-label
        group; stationary = XT chunk, moving = W labels)
  ACT:  exp_g = Exp(scoresT + mask_bias)          (bias per-partition = per-s)
  DVE:  tot   = sum_sc exp_g chunks               (3 adds)
  GPS:  totb  = partition_all_reduce(tot)         (softmax denom, bcast)
  DVE:  attn  = exp_g * reciprocal(totb)          (f32r, pre-normalized)
  mm2:  out[l,h] psum = sum_sc attn_sc^T @ XB_sc  (16 MMs per group;
        stationary = attn chunk -- directly, no transpose needed)
  evac: ACT/DVE copy psum->SBUF, DMA to out[b, l-tile, :]

One-group software pipeline (mm2 of group i emitted after mm1 of group
i+1) hides the exp->normalize chain latency behind mm1 of the next
group, so the PE never waits on softmax.

Matmul operands are float32r end-to-end: the DRAM tensors are declared
f32r so DMA lands input bytes directly in the resident SBUF tiles (no
staging casts). A short burst of dummy matmuls at t=0 warms the PE HAM
clock gate (1.2 -> 2.4 GHz) while the first input DMAs stream.
"""

from contextlib import ExitStack

import numpy as np

import concourse.bass as bass
import concourse.mybir as mybir
import concourse.tile as tile
from concourse import bacc, bass_utils
from concourse.bass import bass_isa, ds, ts

F32 = mybir.dt.float32
R32 = mybir.dt.float32r

# Problem shapes (hardcoded per contract).
B, S, H, L = 4, 512, 512, 28415
N_CORES = 8
LSH = 3584               # per-core padded label count
L_PAD = LSH * N_CORES    # 28672


def build_module_c(b_sz=B, s_sz=S, h_sz=H, lsh=LSH, n_devices=N_CORES,
                   n_warm=24):
    P = 128
    KH = h_sz // P   # H contraction chunks (mm1)
    KS = s_sz // P   # S contraction chunks (mm2)
    LG = 512         # label group per pass (PSUM bank limit)
    NG = lsh // LG   # label groups per core
    NSUB = LG // P   # 128-label output tiles per group

    nc = bacc.Bacc(
        "TRN2", target_bir_lowering=False, debug=False, num_devices=n_devices
    )
    x_d = nc.dram_tensor("x", [b_sz, s_sz, h_sz], R32, kind="ExternalInput").ap()
    xt_d = nc.dram_tensor("xt", [b_sz, h_sz, s_sz], R32, kind="ExternalInput").ap()
    wt_d = nc.dram_tensor("wt", [h_sz, lsh], R32, kind="ExternalInput").ap()
    m_d = nc.dram_tensor("m", [b_sz, s_sz], F32, kind="ExternalInput").ap()
    o_d = nc.dram_tensor("o", [b_sz, lsh, h_sz], F32, kind="ExternalOutput").ap()

    with tile.TileContext(nc) as tc, ExitStack() as ctx:
        const = ctx.enter_context(tc.tile_pool(name="const", bufs=1))
        res = ctx.enter_context(tc.tile_pool(name="res", bufs=1))
        work = ctx.enter_context(tc.tile_pool(name="work", bufs=2))
        psum = ctx.enter_context(tc.tile_pool(name="psum", bufs=1, space="PSUM"))

        warm_f = const.tile([P, LG], F32)
        nc.gpsimd.memset(warm_f[:], 0.0)
        warm_w = const.tile([P, P], R32)
        nc.vector.tensor_copy(warm_w[:], warm_f[:, :P])
        warm_x = const.tile([P, LG], R32)
        nc.vector.tensor_copy(warm_x[:], warm_f[:])

        # Resident SBUF tensors; DMA'd straight from DRAM (same bits).
        WT = res.tile([P, KH, lsh], R32)          # WT[h%128, h//128, l] = W[l, h]
        XT = res.tile([P, b_sz, KH, s_sz], R32)   # XT[h%128, b, h//128, s]
        XB = res.tile([P, b_sz, KS, h_sz], R32)   # XB[s%128, b, s//128, h]
        MB = res.tile([P, b_sz, KS], F32)         # exp bias: (mask-1)*30 per s

        def mask_setup():
            mbr = work.tile([P, b_sz, KS], F32, tag="mbr")
            nc.sync.dma_start(mbr[:], m_d.rearrange("b (c p) -> p b c", p=P))
            nc.vector.tensor_scalar_mul(out=mbr[:], in0=mbr[:], scalar1=30.0)
            nc.vector.tensor_scalar_add(out=MB[:], in0=mbr[:], scalar1=-30.0)

        def warmup(n):
            ps_warm = psum.tile([P, LG], F32, tag="ps_out", bufs=4)
            for _ in range(n):
                nc.tensor.matmul(
                    ps_warm[:], warm_w[:], warm_x[:], start=True, stop=True
                )

        # Steady-state prefetches issue from gpsimd so their 1 MB transfers
        # ride a separate DMA queue from the latency-sensitive out/sums DMAs
        # on the sync queue.
        def dma_xt(b, eng=None):
            (eng or nc.gpsimd).dma_start(
                XT[:, b], xt_d[b].rearrange("(k p) s -> p k s", p=P)
            )

        def dma_xb(b, eng=None):
            (eng or nc.gpsimd).dma_start(
                XB[:, b], x_d[b].rearrange("(c p) h -> p c h", p=P)
            )

        def dma_wt(g, eng=None):
            (eng or nc.gpsimd).dma_start(
                WT[:, :, ts(g, LG)],
                wt_d[:, ts(g, LG)].rearrange("(k p) l -> p k l", p=P),
            )

        def mm1(b, g):
            ps_sct = psum.tile([P, KS, LG], F32, tag="ps_sct", bufs=1)
            for sc in range(KS):
                for k in range(KH):
                    nc.tensor.matmul(
                        ps_sct[:, sc, :],
                        XT[:, b, k, ts(sc, P)],
                        WT[:, k, ts(g, LG)],
                        start=(k == 0),
                        stop=(k == KH - 1),
                    )
            return ps_sct

        def exp_front(b, g, ps_sct):
            """exp into f32r -- directly the mm2 stationary."""
            exp_g = work.tile([P, KS, LG], R32, tag="exp", bufs=3)
            for sc in range(KS):
                nc.scalar.activation(
                    exp_g[:, sc, :], ps_sct[:, sc, :],
                    mybir.ActivationFunctionType.Exp,
                    bias=MB[:, b, sc : sc + 1],
                )
            return exp_g

        def rowsum(exp_g):
            """Label-sums: DVE adds + gpsimd partition reduce, then a tiny
            SBUF->SBUF DMA lays the row out as a per-label column. Two
            pipeline stages of slack before mm2's evac scale needs it."""
            tot = work.tile([P, LG], F32, tag="tot", bufs=2)
            nc.vector.tensor_tensor(
                out=tot[:], in0=exp_g[:, 0, :], in1=exp_g[:, 1, :],
                op=mybir.AluOpType.add,
            )
            nc.vector.tensor_tensor(
                out=tot[:], in0=tot[:], in1=exp_g[:, 2, :],
                op=mybir.AluOpType.add,
            )
            nc.vector.tensor_tensor(
                out=tot[:], in0=tot[:], in1=exp_g[:, 3, :],
                op=mybir.AluOpType.add,
            )
            nc.gpsimd.partition_all_reduce(
                tot[:], tot[:], channels=P, reduce_op=bass_isa.ReduceOp.add
            )
            sums_col = work.tile([P, NSUB], F32, tag="sums_col", bufs=2)
            for l in range(NSUB):
                nc.sync.dma_start(sums_col[:, l : l + 1], tot[0:1, ts(l, P)])
            return sums_col

        def mm2(b, g, exp_g, sums_col):
            recips = work.tile([P, NSUB], F32, tag="recips", bufs=2)
            nc.vector.reciprocal(recips[:], sums_col[:])

            for l in range(NSUB):
                ps_out = psum.tile([P, h_sz], F32, tag="ps_out", bufs=4)
                for sc in range(KS):
                    nc.tensor.matmul(
                        ps_out[:],
                        exp_g[:, sc, ts(l, P)],
                        XB[:, b, sc, :],
                        start=(sc == 0),
                        stop=(sc == KS - 1),
                    )
                out_t = work.tile([P, h_sz], F32, tag="out", bufs=6)
                # ACT-evacuated tiles go out on the scalar-issued DMA queue,
                # DVE-evacuated ones on the sync queue: no single DMA FIFO
                # gets head-of-line blocked by a late producer.
                if l < 2:
                    nc.scalar.activation(
                        out_t[:], ps_out[:], mybir.ActivationFunctionType.Copy,
                        scale=recips[:, l : l + 1],
                    )
                else:
                    nc.vector.tensor_scalar_mul(
                        out=out_t[:], in0=ps_out[:],
                        scalar1=recips[:, l : l + 1],
                    )
                nc.sync.dma_start(o_d[b, ds(g * LG + l * P, P), :], out_t[:])

        warmup(n_warm)
        mask_setup()
        dma_xt(0, eng=nc.sync)
        dma_wt(0, eng=nc.sync)
        dma_wt(1, eng=nc.sync)
        dma_xb(0, eng=nc.sync)
        pend = []
        for b in range(b_sz):
            for g in range(NG):
                if b == 0 and g + 2 < NG:
                    dma_wt(g + 2)
                if g == 3 and b + 1 < b_sz:
                    dma_xt(b + 1)
                    dma_xb(b + 1)
                ps = mm1(b, g)
                exp_g = exp_front(b, g, ps)
                if len(pend) == 2:
                    mm2(*pend.pop(0))
                sums_col = rowsum(exp_g)
                pend.append((b, g, exp_g, sums_col))
        for args in pend:
            mm2(*args)

    nc.compile()
    return nc


_CACHE = {}

VARIANT = "c"


def _get_module():
    if VARIANT not in _CACHE:
        _CACHE[VARIANT] = build_module_c()
    return _CACHE[VARIANT]


def _run(inputs: np.ndarray, masks: np.ndarray, W: np.ndarray, **spmd_kwargs):
    """Run on 8 cores; returns (full output, BassKernelResults)."""
    nc = _get_module()

    x = np.ascontiguousarray(inputs, dtype=np.float32)
    xt = np.ascontiguousarray(np.swapaxes(x, 1, 2))
    mf = np.ascontiguousarray(masks, dtype=np.float32)
    wt_pad = np.zeros((H, L_PAD), dtype=np.float32)
    wt_pad[:, :L] = W.T

    in_maps = [
        {
            "x": x,
            "xt": xt,
            "m": mf,
            "wt": np.ascontiguousarray(wt_pad[:, c * LSH : (c + 1) * LSH]),
        }
        for c in range(N_CORES)
    ]
    res = bass_utils.run_bass_kernel_spmd(
        nc, in_maps, core_ids=list(range(N_CORES)), **spmd_kwargs
    )
    out = np.concatenate([res.results[c]["o"] for c in range(N_CORES)], axis=1)
    return np.ascontiguousarray(out[:, :L, :]), res


def kernel(inputs: np.ndarray, masks: np.ndarray, W: np.ndarray) -> np.ndarray:
    out, _ = _run(inputs, masks, W)
    return out


# revision 26
# speedup vs baseline: 1.1261x; 1.1216x over previous
"""MLAttention (label-pooling attention) Trainium2 Bass kernel.

Computes, for full inputs:
    scores = einsum('bsh,lh->bls', inputs, W)
    scores = where(mask==0, -inf, scores)
    attn   = softmax(scores, axis=-1)
    out    = einsum('bls,bsh->blh', attn, inputs)

Label-parallel across 8 NeuronCores: L=28415 padded to 28672 = 8*3584.
Each core gets its own W shard [3584, 512]; inputs/masks replicated.
Host concatenates the 8 per-core outputs [B, 3584, H] and trims to L.

Variant "c" (current): transposed-scores dataflow sized so the PE runs
*only* the 896 essential N=512 matmuls per core (no transposes, no
rowsum matmuls):

  mm1:  scoresT[s,l] psum = sum_k XT_k^T @ WT_k   (16 MMs per 512-label
        group; stationary = XT chunk, moving = W labels)
  ACT:  exp_g = Exp(scoresT + mask_bias)          (bias per-partition = per-s)
  DVE:  tot   = sum_sc exp_g chunks               (3 adds)
  GPS:  totb  = partition_all_reduce(tot)         (softmax denom, bcast)
  DVE:  attn  = exp_g * reciprocal(totb)          (f32r, pre-normalized)
  mm2:  out[l,h] psum = sum_sc attn_sc^T @ XB_sc  (16 MMs per group;
        stationary = attn chunk -- directly, no transpose needed)
  evac: ACT/DVE copy psum->SBUF, DMA to out[b, l-tile, :]

One-group software pipeline (mm2 of group i emitted after mm1 of group
i+1) hides the exp->normalize chain latency behind mm1 of the next
group, so the PE never waits on softmax.

Matmul operands are float32r end-to-end: the DRAM tensors are declared
f32r so DMA lands input bytes directly in the resident SBUF tiles (no
staging casts). A short burst of dummy matmuls at t=0 warms the PE HAM
clock gate (1.2 -> 2.4 GHz) while the first input DMAs stream.
"""

from contextlib import ExitStack

import numpy as np

import concourse.bass as bass
import concourse.mybir as mybir
import concourse.tile as tile
from concourse import bacc, bass_utils
from concourse.bass import bass_isa, ds, ts

F32 = mybir.dt.float32
R32 = mybir.dt.float32r

# Problem shapes (hardcoded per contract).
B, S, H, L = 4, 512, 512, 28415
N_CORES = 8
LSH = 3584               # per-core padded label count
L_PAD = LSH * N_CORES    # 28672


def build_module_c(b_sz=B, s_sz=S, h_sz=H, lsh=LSH, n_devices=N_CORES,
                   n_warm=24):
    P = 128
    KH = h_sz // P   # H contraction chunks (mm1)
    KS = s_sz // P   # S contraction chunks (mm2)
    LG = 512         # label group per pass (PSUM bank limit)
    NG = lsh // LG   # label groups per core
    NSUB = LG // P   # 128-label output tiles per group

    nc = bacc.Bacc(
        "TRN2", target_bir_lowering=False, debug=False, num_devices=n_devices
    )
    x_d = nc.dram_tensor("x", [b_sz, s_sz, h_sz], R32, kind="ExternalInput").ap()
    xt_d = nc.dram_tensor("xt", [b_sz, h_sz, s_sz], R32, kind="ExternalInput").ap()
    wt_d = nc.dram_tensor("wt", [h_sz, lsh], R32, kind="ExternalInput").ap()
    m_d = nc.dram_tensor("m", [b_sz, s_sz], F32, kind="ExternalInput").ap()
    o_d = nc.dram_tensor("o", [b_sz, lsh, h_sz], F32, kind="ExternalOutput").ap()

    with tile.TileContext(nc) as tc, ExitStack() as ctx:
        const = ctx.enter_context(tc.tile_pool(name="const", bufs=1))
        res = ctx.enter_context(tc.tile_pool(name="res", bufs=1))
        work = ctx.enter_context(tc.tile_pool(name="work", bufs=2))
        psum = ctx.enter_context(tc.tile_pool(name="psum", bufs=1, space="PSUM"))

        warm_f = const.tile([P, LG], F32)
        nc.gpsimd.memset(warm_f[:], 0.0)
        warm_w = const.tile([P, P], R32)
        nc.vector.tensor_copy(warm_w[:], warm_f[:, :P])
        warm_x = const.tile([P, LG], R32)
        nc.vector.tensor_copy(warm_x[:], warm_f[:])

        # Resident SBUF tensors; DMA'd straight from DRAM (same bits).
        WT = res.tile([P, KH, lsh], R32)          # WT[h%128, h//128, l] = W[l, h]
        XT = res.tile([P, b_sz, KH, s_sz], R32)   # XT[h%128, b, h//128, s]
        XB = res.tile([P, b_sz, KS, h_sz], R32)   # XB[s%128, b, s//128, h]
        MB = res.tile([P, b_sz, KS], F32)         # exp bias: (mask-1)*30 per s

        def mask_setup():
            mbr = work.tile([P, b_sz, KS], F32, tag="mbr")
            nc.sync.dma_start(mbr[:], m_d.rearrange("b (c p) -> p b c", p=P))
            nc.vector.tensor_scalar_mul(out=mbr[:], in0=mbr[:], scalar1=30.0)
            nc.vector.tensor_scalar_add(out=MB[:], in0=mbr[:], scalar1=-30.0)

        def warmup(n):
            ps_warm = psum.tile([P, LG], F32, tag="ps_out", bufs=4)
            for _ in range(n):
                nc.tensor.matmul(
                    ps_warm[:], warm_w[:], warm_x[:], start=True, stop=True
                )

        def dma_xt(b):
            nc.sync.dma_start(XT[:, b], xt_d[b].rearrange("(k p) s -> p k s", p=P))

        def dma_xb(b):
            nc.sync.dma_start(XB[:, b], x_d[b].rearrange("(c p) h -> p c h", p=P))

        # 256 KB prefetch chunks: spread batch b's inputs over 4 groups so
        # the transfers interleave with out/sums DMAs on the sync queue
        # instead of monopolizing it for ~6 us at batch transitions.
        def dma_xt_chunk(b, k):
            nc.sync.dma_start(XT[:, b, k], xt_d[b, ts(k, P), :])

        def dma_xb_chunk(b, c):
            nc.sync.dma_start(XB[:, b, c], x_d[b, ts(c, P), :])

        def dma_wt(g):
            nc.sync.dma_start(
                WT[:, :, ts(g, LG)],
                wt_d[:, ts(g, LG)].rearrange("(k p) l -> p k l", p=P),
            )

        def mm1(b, g):
            ps_sct = psum.tile([P, KS, LG], F32, tag="ps_sct", bufs=1)
            for sc in range(KS):
                for k in range(KH):
                    nc.tensor.matmul(
                        ps_sct[:, sc, :],
                        XT[:, b, k, ts(sc, P)],
                        WT[:, k, ts(g, LG)],
                        start=(k == 0),
                        stop=(k == KH - 1),
                    )
            return ps_sct

        def exp_front(b, g, ps_sct):
            """exp into f32r -- directly the mm2 stationary."""
            exp_g = work.tile([P, KS, LG], R32, tag="exp", bufs=3)
            for sc in range(KS):
                nc.scalar.activation(
                    exp_g[:, sc, :], ps_sct[:, sc, :],
                    mybir.ActivationFunctionType.Exp,
                    bias=MB[:, b, sc : sc + 1],
                )
            return exp_g

        def rowsum(exp_g):
            """Label-sums: DVE adds + gpsimd partition reduce, then a tiny
            SBUF->SBUF DMA lays the row out as a per-label column. Two
            pipeline stages of slack before mm2's evac scale needs it."""
            tot = work.tile([P, LG], F32, tag="tot", bufs=2)
            nc.vector.tensor_tensor(
                out=tot[:], in0=exp_g[:, 0, :], in1=exp_g[:, 1, :],
                op=mybir.AluOpType.add,
            )
            nc.vector.tensor_tensor(
                out=tot[:], in0=tot[:], in1=exp_g[:, 2, :],
                op=mybir.AluOpType.add,
            )
            nc.vector.tensor_tensor(
                out=tot[:], in0=tot[:], in1=exp_g[:, 3, :],
                op=mybir.AluOpType.add,
            )
            nc.gpsimd.partition_all_reduce(
                tot[:], tot[:], channels=P, reduce_op=bass_isa.ReduceOp.add
            )
            sums_col = work.tile([P, NSUB], F32, tag="sums_col", bufs=2)
            for l in range(NSUB):
                nc.sync.dma_start(sums_col[:, l : l + 1], tot[0:1, ts(l, P)])
            return sums_col

        def mm2(b, g, exp_g, sums_col):
            recips = work.tile([P, NSUB], F32, tag="recips", bufs=2)
            nc.vector.reciprocal(recips[:], sums_col[:])

            for l in range(NSUB):
                ps_out = psum.tile([P, h_sz], F32, tag="ps_out", bufs=4)
                for sc in range(KS):
                    nc.tensor.matmul(
                        ps_out[:],
                        exp_g[:, sc, ts(l, P)],
                        XB[:, b, sc, :],
                        start=(sc == 0),
                        stop=(sc == KS - 1),
                    )
                out_t = work.tile([P, h_sz], F32, tag="out", bufs=6)
                # ACT-evacuated tiles go out on the scalar-issued DMA queue,
                # DVE-evacuated ones on the sync queue: no single DMA FIFO
                # gets head-of-line blocked by a late producer.
                if l < 2:
                    nc.scalar.activation(
                        out_t[:], ps_out[:], mybir.ActivationFunctionType.Copy,
                        scale=recips[:, l : l + 1],
                    )
                else:
                    nc.vector.tensor_scalar_mul(
                        out=out_t[:], in0=ps_out[:],
                        scalar1=recips[:, l : l + 1],
                    )
                nc.sync.dma_start(o_d[b, ds(g * LG + l * P, P), :], out_t[:])

        warmup(n_warm)
        mask_setup()
        dma_xt(0)
        dma_wt(0)
        dma_wt(1)
        dma_xb(0)
        pend = []
        for b in range(b_sz):
            for g in range(NG):
                if b == 0 and g + 2 < NG:
                    dma_wt(g + 2)
                if 1 <= g <= 4 and b + 1 < b_sz:
                    dma_xt_chunk(b + 1, g - 1)
                    dma_xb_chunk(b + 1, g - 1)
                ps = mm1(b, g)
                exp_g = exp_front(b, g, ps)
                if len(pend) == 2:
                    mm2(*pend.pop(0))
                sums_col = rowsum(exp_g)
                pend.append((b, g, exp_g, sums_col))
        for args in pend:
            mm2(*args)

    nc.compile()
    return nc


_CACHE = {}

VARIANT = "c"


def _get_module():
    if VARIANT not in _CACHE:
        _CACHE[VARIANT] = build_module_c()
    return _CACHE[VARIANT]


def _run(inputs: np.ndarray, masks: np.ndarray, W: np.ndarray, **spmd_kwargs):
    """Run on 8 cores; returns (full output, BassKernelResults)."""
    nc = _get_module()

    x = np.ascontiguousarray(inputs, dtype=np.float32)
    xt = np.ascontiguousarray(np.swapaxes(x, 1, 2))
    mf = np.ascontiguousarray(masks, dtype=np.float32)
    wt_pad = np.zeros((H, L_PAD), dtype=np.float32)
    wt_pad[:, :L] = W.T

    in_maps = [
        {
            "x": x,
            "xt": xt,
            "m": mf,
            "wt": np.ascontiguousarray(wt_pad[:, c * LSH : (c + 1) * LSH]),
        }
        for c in range(N_CORES)
    ]
    res = bass_utils.run_bass_kernel_spmd(
        nc, in_maps, core_ids=list(range(N_CORES)), **spmd_kwargs
    )
    out = np.concatenate([res.results[c]["o"] for c in range(N_CORES)], axis=1)
    return np.ascontiguousarray(out[:, :L, :]), res


def kernel(inputs: np.ndarray, masks: np.ndarray, W: np.ndarray) -> np.ndarray:
    out, _ = _run(inputs, masks, W)
    return out
